# revision 26
# baseline (speedup 1.0000x reference)
"""Trainium2 Bass kernel for BertEmbedding segment-mean-pool + linear.

Reference computation (per batch element b):
    pooled[t, :] = mean_{s : word_ids[b,s]==t} hidden[b, s, :]   (0 if empty)
    pooled[t, :] = 0 where t >= token_lengths[b]
    out[b] = pooled @ W.T + b_bias                                [T, E]

Shapes: hidden [64, 512, 768] f32, word_ids [64, 512] i32 (sorted),
token_lengths [64] i32, W [512, 768] f32, b [512] f32 -> out [64, 256, 512].

Strategy: data-parallel over batch across 8 NeuronCores (8 sentences/core).
Per sentence, on device:
  1. one-hot oh[s, t] = (word_ids[s] == t) built via iota + is_equal (DVE)
  2. pooled_sums^T [h, t] = hidden^T-contraction via PE matmul
     (stationary = hidden tile [s, h-chunk], moving = one-hot [s, t]),
     counts [1, t] via ones-vector stationary.  fp32r (FP22) full-rate.
  3. counts computed directly in column form (oh chunk as stationary,
     two ones columns moving -> counts^T [128t, 2] per c-chunk), so
     scale[t] = (t < len) / max(counts, 1) is three small per-partition
     DVE ops (no row->column transpose DMAs); word_ids for all 8
     sentences loaded/converted once before the loop
  4. pooled^T moved PSUM->SBUF by merged pair copies (ACT/DVE/ACT)
  5. out[t-chunk] [128t, 512e] = sum_h pooled^T[h, tc]^T @ W^T[h, e]
  6. scale+bias fused in one DVE scalar_tensor_tensor
     (psum * scol + bias_bc; scale commutes through the linear),
     DMA out rows (2KB contiguous per partition)
  7. hidden loads and output stores are batched per sentence PAIR (one
     256x12KB-descriptor load, one 512x2KB-descriptor store per pair,
     load and store on opposite HWDGE rings, rings alternating per
     pair) to minimize DGE issue/semaphore overhead; bufs=3 and
     psum_bufs=2 deepen the cross-sentence pipeline
"""

import sys

if "/opt/trn_rl_repo" not in sys.path:
    sys.path.insert(0, "/opt/trn_rl_repo")

import numpy as np

B, S, H, E, T = 64, 512, 768, 512, 256
NCORES = 8
BL = B // NCORES  # sentences per core
KS = S // 128  # 4 s-tiles (contraction of matmul 1)
KH = H // 128  # 6 h-tiles (contraction of matmul 2)
CT = T // 128  # 2 t-chunks of the output

_cache: dict = {}


def _build(reps: int = 1, bufs: int = 3, psum_bufs: int = 2, dma2: int = 1, dma3: int = 0, widfirst: int = 0, outmerge: int = 0, dma4: int = 0, ptsdve: int = 0, v2: int = 1, ohpool: int = 0, dmav: int = 0, pairld: int = 1, ablate: str = ""):
    """Build + compile the per-core Bass program. Returns the Bacc object.

    reps > 1 repeats the whole per-core computation (used only for timing
    slope measurements in test.py). ablate: comma list of {mm1,mm2,cnt}
    for timing ablations (breaks correctness)."""
    ablated = set(ablate.split(",")) if ablate else set()
    from concourse import bacc, tile, mybir

    f32 = mybir.dt.float32
    f32r = mybir.dt.float32r
    i32 = mybir.dt.int32
    Alu = mybir.AluOpType
    Act = mybir.ActivationFunctionType

    nc = bacc.Bacc("TRN2", target_bir_lowering=False, debug=False, num_devices=NCORES)

    h_d = nc.dram_tensor("h", [BL, S, H], f32r, kind="ExternalInput")
    wid_d = nc.dram_tensor("wid", [BL, S], i32, kind="ExternalInput")
    tl_d = nc.dram_tensor("tl", [1, BL], i32, kind="ExternalInput")
    wt_d = nc.dram_tensor("wt", [H, E], f32r, kind="ExternalInput")  # W^T
    bias_d = nc.dram_tensor("bias", [1, E], f32r, kind="ExternalInput")
    ones_d = nc.dram_tensor("ones", [1, 128], f32r, kind="ExternalInput")
    out_d = nc.dram_tensor("out", [BL, T, E], f32, kind="ExternalOutput")

    with tile.TileContext(nc) as tc:
        with (
            tc.tile_pool(name="const", bufs=1) as cpool,
            tc.tile_pool(name="work", bufs=bufs) as wpool,
            tc.tile_pool(name="psum", bufs=1, space="PSUM") as ppool,
        ):
            # ---- one-time constants ----
            if "ld" in ablated:
                hs_const = cpool.tile([128, KS, H], f32r, tag="hsc")
                nc.vector.memset(hs_const[:].bitcast(f32), 0.25)
            iota_i = cpool.tile([128, T], i32)
            nc.gpsimd.iota(iota_i[:], pattern=[[1, T]], base=0, channel_multiplier=0)
            iota_f = cpool.tile([128, T], f32)
            nc.vector.tensor_copy(iota_f[:], iota_i[:])
            ones_col = cpool.tile([128, 1], f32r)
            nc.sync.dma_start(ones_col[:], ones_d[0].rearrange("(p o) -> p o", o=1))
            tl_i = cpool.tile([1, BL], i32)
            nc.sync.dma_start(tl_i[:], tl_d[:])
            tl_f = cpool.tile([1, BL], f32)
            nc.vector.tensor_copy(tl_f[:], tl_i[:])
            wt_t = cpool.tile([128, KH, E], f32r)
            nc.sync.dma_start(wt_t[:], wt_d[:, :].rearrange("(k p) e -> p k e", p=128))
            bias_row = cpool.tile([1, E], f32r)
            nc.sync.dma_start(bias_row[:], bias_d[:])
            b_bc = cpool.tile([128, E], f32)
            nc.gpsimd.partition_broadcast(b_bc[:], bias_row[:].bitcast(f32))
            if v2:
                # iota over t in column form: iota_col[p, c] = p + 128*c
                iota_col_i = cpool.tile([128, CT], i32)
                nc.gpsimd.iota(
                    iota_col_i[:], pattern=[[128, CT]], base=0, channel_multiplier=1
                )
                iota_col = cpool.tile([128, CT], f32)
                nc.vector.tensor_copy(iota_col[:], iota_col_i[:])
                # token_lengths broadcast to all partitions: [128, BL]
                tl_bc = cpool.tile([128, BL], f32)
                nc.gpsimd.partition_broadcast(tl_bc[:], tl_f[:])
                # all sentences' word ids in one load: s = 4p + k
                wid_all = cpool.tile([128, BL, KS], i32)
                nc.sync.dma_start(
                    wid_all[:], wid_d[:, :].rearrange("b (p k) -> p b k", k=KS)
                )
                wid_f_all = cpool.tile([128, BL, KS], f32)
                nc.vector.tensor_copy(wid_f_all[:], wid_all[:])
                # two ones columns: fp32r matmul needs moving free size >= 2
                ones2 = cpool.tile([128, 2], f32r)
                nc.vector.memset(ones2[:].bitcast(f32), 1.0)

            for i in range(BL * reps):
                i = i % BL
                # ---- load sentence: s is laid out as s = 4*p + k ----
                if widfirst and not v2:
                    wid_t = wpool.tile([128, KS], i32, tag="wid")
                    nc.sync.dma_start(wid_t[:], wid_d[i].rearrange("(p k) -> p k", k=KS))
                if pairld:
                    # one load per sentence PAIR: 256 descriptors of 12KB,
                    # rings alternate per pair
                    if i % 2 == 0:
                        hs2 = wpool.tile([128, 2, KS, H], f32r, tag="hs2")
                        outs2 = wpool.tile([128, 2, CT, E], f32, tag="outs2")
                        ldeng = nc.sync if (i // 2) % 2 == 0 else nc.scalar
                        ldeng.dma_start(
                            hs2[:],
                            h_d[i : i + 2].rearrange("b (p k) c -> p b k c", k=KS),
                        )
                    hs = hs2[:, i % 2]
                if pairld:
                    pass
                elif "ld" in ablated:
                    hs = hs_const
                else:
                    hs = wpool.tile([128, KS, H], f32r, tag="hs")
                h_src = h_d[i].rearrange("(p k) c -> p k c", k=KS)
                if pairld or "ld" in ablated:
                    pass
                elif dma3:
                    nc.sync.dma_start(hs[:, 0:2, :], h_src[:, 0:2, :])
                    nc.scalar.dma_start(hs[:, 2:3, :], h_src[:, 2:3, :])
                    nc.gpsimd.dma_start(hs[:, 3:4, :], h_src[:, 3:4, :])
                elif dma4:
                    nc.sync.dma_start(hs[:, 0:1, :], h_src[:, 0:1, :])
                    nc.scalar.dma_start(hs[:, 2:3, :], h_src[:, 2:3, :])
                    nc.sync.dma_start(hs[:, 1:2, :], h_src[:, 1:2, :])
                    nc.scalar.dma_start(hs[:, 3:4, :], h_src[:, 3:4, :])
                elif dma2:
                    nc.sync.dma_start(hs[:, 0:2, :], h_src[:, 0:2, :])
                    nc.scalar.dma_start(hs[:, 2:4, :], h_src[:, 2:4, :])
                else:
                    nc.sync.dma_start(hs[:], h_src)
                if not widfirst and not v2:
                    wid_t = wpool.tile([128, KS], i32, tag="wid")
                    nc.sync.dma_start(wid_t[:], wid_d[i].rearrange("(p k) -> p k", k=KS))
                if not v2:
                    wid_f = wpool.tile([128, KS], f32, tag="widf")
                    nc.vector.tensor_copy(wid_f[:], wid_t[:])

                # ---- one-hot [s, t] ----
                oh = wpool.tile([128, KS, T], f32r, tag="oh")
                for k in range(KS):
                    wsrc = wid_f_all[:, i, k : k + 1] if v2 else wid_f[:, k : k + 1]
                    eng = nc.gpsimd if k < ohpool else nc.vector
                    eng.tensor_scalar(
                        oh[:, k, :], iota_f[:], wsrc, None, Alu.is_equal
                    )

                if v2 == 1:
                    # ---- counts^T[t, 1] per c-chunk, straight to column form ----
                    counts_ps = ppool.tile([128, CT, 2], f32, tag="cnt")
                    if "cnt" in ablated:
                        nc.vector.memset(counts_ps[:], 2.0)
                    for c in range(CT if "cnt" not in ablated else 0):
                        for k in range(KS):
                            nc.tensor.matmul(
                                counts_ps[:, c, :],
                                oh[:, k, c * 128 : (c + 1) * 128],
                                ones2[:],
                                start=(k == 0),
                                stop=(k == KS - 1),
                            )
                    # ---- scol[t] = (t < len) / max(counts, 1), per-partition ----
                    crec = wpool.tile([128, CT], f32, tag="crec")
                    nc.vector.tensor_scalar(
                        crec[:], counts_ps[:, :, 0], 1.0, None, Alu.max
                    )
                    nc.vector.reciprocal(crec[:], crec[:])
                    scol = wpool.tile([128, CT], f32, tag="scol")
                    nc.vector.scalar_tensor_tensor(
                        scol[:], iota_col[:], tl_bc[:, i : i + 1], crec[:],
                        Alu.is_lt, Alu.mult,
                    )
                else:
                    # ---- counts[1, t] = sum_s oh[s, t] ----
                    counts_ps = ppool.tile([1, T], f32, tag="cnt")
                    if "cnt" in ablated:
                        nc.vector.memset(counts_ps[:], 2.0)
                    for k in range(KS if "cnt" not in ablated else 0):
                        nc.tensor.matmul(
                            counts_ps[:],
                            ones_col[:],
                            oh[:, k, :],
                            start=(k == 0),
                            stop=(k == KS - 1),
                        )

                    # ---- scale[t] = (t < len) / max(counts, 1) ----
                    cmax = wpool.tile([1, T], f32, tag="cmax")
                    nc.vector.tensor_scalar(cmax[:], counts_ps[:], 1.0, None, Alu.max)
                    crec = wpool.tile([1, T], f32, tag="crec")
                    nc.vector.reciprocal(crec[:], cmax[:])
                    mask = wpool.tile([1, T], f32, tag="mask")
                    nc.vector.tensor_scalar(
                        mask[:], iota_f[0:1, :], tl_f[0:1, i : i + 1], None, Alu.is_lt
                    )
                    srow = wpool.tile([1, T], f32, tag="srow")
                    nc.vector.tensor_tensor(srow[:], crec[:], mask[:], Alu.mult)
                    # transpose scale_row -> per-partition scale columns [128, CT]
                    scol = wpool.tile([128, CT], f32, tag="scol")
                    scol_eng = nc.scalar if widfirst else nc.sync
                    for c in range(CT):
                        scol_eng.dma_start(
                            scol[:, c : c + 1],
                            srow[0:1, c * 128 : (c + 1) * 128],
                        )

                # ---- matmul 1: pooled_sums^T [h, t] ----
                pt_ps = [
                    ppool.tile([128, 2 * T], f32, name=f"pt{j}", tag=f"pt{j}")
                    for j in range(3)
                ]
                for m in range(KH if "mm1" not in ablated else 0):
                    dst = pt_ps[m // 2][:, (m % 2) * T : (m % 2 + 1) * T]
                    for k in range(KS):
                        nc.tensor.matmul(
                            dst,
                            hs[:, k, m * 128 : (m + 1) * 128],
                            oh[:, k, :],
                            start=(k == 0),
                            stop=(k == KS - 1),
                        )

                # ---- move to SBUF (plain copies, split ACT/DVE) ----
                pts = wpool.tile([128, KH, T], f32r, tag="pts")
                if "mm1" in ablated:
                    for j in range(3):
                        nc.vector.memset(pt_ps[j][:], 0.5)
                if v2:
                    # merged pair copies [128, 2T]: ACT, DVE, ACT
                    for j in range(3):
                        src_ap = pt_ps[j][:, 0 : 2 * T]
                        dst_ap = pts[:, 2 * j : 2 * j + 2, :]
                        if j == 1:
                            nc.vector.tensor_copy(dst_ap, src_ap)
                        else:
                            nc.scalar.copy(dst_ap, src_ap)
                else:
                    for m in range(KH):
                        src_ap = pt_ps[m // 2][:, (m % 2) * T : (m % 2 + 1) * T]
                        if m % 2 == 0 and not ptsdve:
                            nc.scalar.copy(pts[:, m, :], src_ap)
                        else:
                            nc.vector.tensor_copy(pts[:, m, :], src_ap)

                # ---- matmul 2: out[t, e] = pooled @ W^T ----
                out_ps = [
                    ppool.tile([128, E], f32, name=f"o2{c}", tag=f"o2{c}", bufs=psum_bufs)
                    for c in range(CT)
                ]
                if "mm2" in ablated:
                    for c in range(CT):
                        nc.vector.memset(out_ps[c][:], 0.25)
                for c in range(CT if "mm2" not in ablated else 0):
                    for k in range(KH):
                        nc.tensor.matmul(
                            out_ps[c][:],
                            pts[:, k, c * 128 : (c + 1) * 128],
                            wt_t[:, k, :],
                            start=(k == 0),
                            stop=(k == KH - 1),
                        )

                # ---- scale (per-partition), add bias, PSUM -> SBUF -> DRAM ----
                if pairld:
                    outs = outs2[:, i % 2]
                else:
                    outs = wpool.tile([128, CT, E], f32, tag="outs")
                if v2:
                    for c in range(CT):
                        nc.vector.scalar_tensor_tensor(
                            outs[:, c, :], out_ps[c][:], scol[:, c : c + 1], b_bc[:],
                            Alu.mult, Alu.add,
                        )
                else:
                    for c in range(CT):
                        nc.scalar.activation(
                            outs[:, c, :],
                            out_ps[c][:],
                            Act.Copy,
                            scale=scol[:, c : c + 1],
                        )
                        nc.vector.tensor_tensor(outs[:, c, :], outs[:, c, :], b_bc[:], Alu.add)
                if "st" in ablated:
                    pass
                elif pairld:
                    # one store per pair: 512 descriptors of 2KB, opposite
                    # ring from the pair's load
                    if i % 2 == 1:
                        steng = nc.scalar if (i // 2) % 2 == 0 else nc.sync
                        steng.dma_start(
                            out_d[i - 1 : i + 1].rearrange(
                                "b (c p) e -> p b c e", c=CT
                            ),
                            outs2[:],
                        )
                elif outmerge:
                    eng = nc.scalar if i % 2 else nc.sync
                    eng.dma_start(
                        out_d[i].rearrange("(c p) e -> p c e", c=CT), outs[:, :, :]
                    )
                else:
                    for c in range(CT):
                        if dmav:
                            eng = nc.vector if c == 0 else nc.scalar
                        else:
                            eng = nc.scalar if (dma2 and c == 1) else nc.sync
                        eng.dma_start(out_d[i, c * 128 : (c + 1) * 128, :], outs[:, c, :])

    nc.compile()
    return nc


def _get_nc(reps: int = 1, **opts):
    key = f"nc{reps}|{sorted(opts.items())}"
    if key not in _cache:
        _cache[key] = _build(reps, **opts)
    return _cache[key]


def _in_maps(hidden_states, word_ids, token_lengths, W, b):
    wt = np.ascontiguousarray(W.T.astype(np.float32, copy=False))
    bias = np.ascontiguousarray(b.astype(np.float32, copy=False)).reshape(1, E)
    maps = []
    for c in range(NCORES):
        sl = slice(c * BL, (c + 1) * BL)
        maps.append(
            {
                "h": np.ascontiguousarray(hidden_states[sl]).astype(np.float32, copy=False),
                "wid": np.ascontiguousarray(word_ids[sl]).astype(np.int32, copy=False),
                "tl": np.ascontiguousarray(token_lengths[sl]).astype(np.int32, copy=False).reshape(1, BL),
                "wt": wt,
                "bias": bias,
                "ones": np.ones((1, 128), np.float32),
            }
        )
    return maps


def kernel(hidden_states, word_ids, token_lengths, W, b):
    from concourse import bass_utils

    nc = _get_nc()
    maps = _in_maps(hidden_states, word_ids, token_lengths, W, b)
    res = bass_utils.run_bass_kernel_spmd(nc, maps, core_ids=list(range(NCORES)))
    out = np.concatenate([res.results[c]["out"] for c in range(NCORES)], axis=0)
    return out



# revision 46
# speedup vs baseline: 5.2910x; 5.2910x over previous
"""Trainium2 Bass kernel for BertEmbedding segment-mean-pool + linear.

Reference computation (per batch element b):
    pooled[t, :] = mean_{s : word_ids[b,s]==t} hidden[b, s, :]   (0 if empty)
    pooled[t, :] = 0 where t >= token_lengths[b]
    out[b] = pooled @ W.T + b_bias                                [T, E]

Shapes: hidden [64, 512, 768] f32, word_ids [64, 512] i32 (sorted),
token_lengths [64] i32, W [512, 768] f32, b [512] f32 -> out [64, 256, 512].

Strategy: data-parallel over batch across 8 NeuronCores (8 sentences/core).
Per sentence, on device:
  1. one-hot oh[s, t] = (word_ids[s] == t) built via iota + is_equal (DVE)
  2. pooled_sums^T [h, t] = hidden^T-contraction via PE matmul
     (stationary = hidden tile [s, h-chunk], moving = one-hot [s, t]),
     counts [1, t] via ones-vector stationary.  fp32r (FP22) full-rate.
  3. counts computed directly in column form (oh chunk as stationary,
     two ones columns moving -> counts^T [128t, 2] per c-chunk), so
     scale[t] = (t < len) / max(counts, 1) is three small per-partition
     DVE ops (no row->column transpose DMAs); word_ids for all 8
     sentences loaded/converted once before the loop
  4. pooled^T moved PSUM->SBUF by merged pair copies (ACT/DVE/ACT)
  5. out[t-chunk] [128t, 512e] = sum_h pooled^T[h, tc]^T @ W^T[h, e]
  6. scale+bias fused in one DVE scalar_tensor_tensor
     (psum * scol + bias_bc; scale commutes through the linear),
     DMA out rows (2KB contiguous per partition)
  7. hidden loads and output stores are batched per sentence PAIR (one
     256x12KB-descriptor load per pair, load and store on opposite
     HWDGE rings, rings alternating per pair) to minimize DGE
     issue/semaphore overhead; bufs=3 and psum_bufs=2 deepen the
     cross-sentence pipeline
  8. the t axis is permuted (storage slot c*128+p holds segment
     t = 2p+c, via the iota value patterns) so each partition's two
     output rows are DRAM-adjacent: the pair store is 256 descriptors
     of 4KB instead of 512 of 2KB
"""

import sys

if "/opt/trn_rl_repo" not in sys.path:
    sys.path.insert(0, "/opt/trn_rl_repo")

import numpy as np

B, S, H, E, T = 64, 512, 768, 512, 256
NCORES = 8
BL = B // NCORES  # sentences per core
KS = S // 128  # 4 s-tiles (contraction of matmul 1)
KH = H // 128  # 6 h-tiles (contraction of matmul 2)
CT = T // 128  # 2 t-chunks of the output

_cache: dict = {}


def _build(reps: int = 1, bufs: int = 3, psum_bufs: int = 2, dma2: int = 1, dma3: int = 0, widfirst: int = 0, outmerge: int = 0, dma4: int = 0, ptsdve: int = 0, v2: int = 1, ohpool: int = 0, dmav: int = 0, pairld: int = 1, v3: int = 0, tperm: int = 1, ablate: str = ""):
    """Build + compile the per-core Bass program. Returns the Bacc object.

    reps > 1 repeats the whole per-core computation (used only for timing
    slope measurements in test.py). ablate: comma list of {mm1,mm2,cnt}
    for timing ablations (breaks correctness)."""
    ablated = set(ablate.split(",")) if ablate else set()
    from concourse import bacc, tile, mybir

    f32 = mybir.dt.float32
    f32r = mybir.dt.float32r
    i32 = mybir.dt.int32
    Alu = mybir.AluOpType
    Act = mybir.ActivationFunctionType

    nc = bacc.Bacc("TRN2", target_bir_lowering=False, debug=False, num_devices=NCORES)

    h_d = nc.dram_tensor("h", [BL, S, H], f32r, kind="ExternalInput")
    wid_d = nc.dram_tensor("wid", [BL, S], i32, kind="ExternalInput")
    tl_d = nc.dram_tensor("tl", [1, BL], i32, kind="ExternalInput")
    wt_d = nc.dram_tensor("wt", [H, E], f32r, kind="ExternalInput")  # W^T
    bias_d = nc.dram_tensor("bias", [1, E], f32r, kind="ExternalInput")
    ones_d = nc.dram_tensor("ones", [1, 128], f32r, kind="ExternalInput")
    out_d = nc.dram_tensor("out", [BL, T, E], f32, kind="ExternalOutput")

    with tile.TileContext(nc) as tc:
        with (
            tc.tile_pool(name="const", bufs=1) as cpool,
            tc.tile_pool(name="work", bufs=bufs) as wpool,
            tc.tile_pool(name="psum", bufs=1, space="PSUM") as ppool,
        ):
            # ---- one-time constants ----
            if "ld" in ablated:
                hs_const = cpool.tile([128, KS, H], f32r, tag="hsc")
                nc.vector.memset(hs_const[:].bitcast(f32), 0.25)
            iota_i = cpool.tile([128, T], i32)
            if tperm:
                # storage slot j = c*128 + p holds segment t = 2p + c, so the
                # store's two rows per partition are DRAM-adjacent (4KB desc)
                nc.gpsimd.iota(
                    iota_i[:], pattern=[[1, CT], [2, 128]], base=0,
                    channel_multiplier=0,
                )
            else:
                nc.gpsimd.iota(iota_i[:], pattern=[[1, T]], base=0, channel_multiplier=0)
            iota_f = cpool.tile([128, T], f32)
            nc.vector.tensor_copy(iota_f[:], iota_i[:])
            ones_col = cpool.tile([128, 1], f32r)
            nc.sync.dma_start(ones_col[:], ones_d[0].rearrange("(p o) -> p o", o=1))
            tl_i = cpool.tile([1, BL], i32)
            nc.sync.dma_start(tl_i[:], tl_d[:])
            tl_f = cpool.tile([1, BL], f32)
            nc.vector.tensor_copy(tl_f[:], tl_i[:])
            wt_t = cpool.tile([128, KH, E], f32r)
            nc.sync.dma_start(wt_t[:], wt_d[:, :].rearrange("(k p) e -> p k e", p=128))
            bias_row = cpool.tile([1, E], f32r)
            nc.sync.dma_start(bias_row[:], bias_d[:])
            b_bc = cpool.tile([128, E], f32)
            nc.gpsimd.partition_broadcast(b_bc[:], bias_row[:].bitcast(f32))
            if v2:
                # iota over t in column form: iota_col[p, c] = p + 128*c
                iota_col_i = cpool.tile([128, CT], i32)
                if tperm:
                    nc.gpsimd.iota(
                        iota_col_i[:], pattern=[[1, CT]], base=0, channel_multiplier=2
                    )
                else:
                    nc.gpsimd.iota(
                        iota_col_i[:], pattern=[[128, CT]], base=0, channel_multiplier=1
                    )
                iota_col = cpool.tile([128, CT], f32)
                nc.vector.tensor_copy(iota_col[:], iota_col_i[:])
                # token_lengths broadcast to all partitions: [128, BL]
                tl_bc = cpool.tile([128, BL], f32)
                nc.gpsimd.partition_broadcast(tl_bc[:], tl_f[:])
                # all sentences' word ids in one load: s = 4p + k
                wid_all = cpool.tile([128, BL, KS], i32)
                nc.sync.dma_start(
                    wid_all[:], wid_d[:, :].rearrange("b (p k) -> p b k", k=KS)
                )
                wid_f_all = cpool.tile([128, BL, KS], f32)
                nc.vector.tensor_copy(wid_f_all[:], wid_all[:])
                # two ones columns: fp32r matmul needs moving free size >= 2
                ones2 = cpool.tile([128, 2], f32r)
                nc.vector.memset(ones2[:].bitcast(f32), 1.0)

            for i in range(BL * reps):
                i = i % BL
                # ---- load sentence: s is laid out as s = 4*p + k ----
                if widfirst and not v2:
                    wid_t = wpool.tile([128, KS], i32, tag="wid")
                    nc.sync.dma_start(wid_t[:], wid_d[i].rearrange("(p k) -> p k", k=KS))
                if pairld:
                    # one load per sentence PAIR: 256 descriptors of 12KB,
                    # rings alternate per pair
                    if i % 2 == 0:
                        hs2 = wpool.tile([128, 2, KS, H], f32r, tag="hs2")
                        outs2 = wpool.tile([128, 2, CT, E], f32, tag="outs2")
                        ldeng = nc.sync if (i // 2) % 2 == 0 else nc.scalar
                        ldeng.dma_start(
                            hs2[:],
                            h_d[i : i + 2].rearrange("b (p k) c -> p b k c", k=KS),
                        )
                    hs = hs2[:, i % 2]
                if pairld:
                    pass
                elif "ld" in ablated:
                    hs = hs_const
                else:
                    hs = wpool.tile([128, KS, H], f32r, tag="hs")
                h_src = h_d[i].rearrange("(p k) c -> p k c", k=KS)
                if pairld or "ld" in ablated:
                    pass
                elif dma3:
                    nc.sync.dma_start(hs[:, 0:2, :], h_src[:, 0:2, :])
                    nc.scalar.dma_start(hs[:, 2:3, :], h_src[:, 2:3, :])
                    nc.gpsimd.dma_start(hs[:, 3:4, :], h_src[:, 3:4, :])
                elif dma4:
                    nc.sync.dma_start(hs[:, 0:1, :], h_src[:, 0:1, :])
                    nc.scalar.dma_start(hs[:, 2:3, :], h_src[:, 2:3, :])
                    nc.sync.dma_start(hs[:, 1:2, :], h_src[:, 1:2, :])
                    nc.scalar.dma_start(hs[:, 3:4, :], h_src[:, 3:4, :])
                elif dma2:
                    nc.sync.dma_start(hs[:, 0:2, :], h_src[:, 0:2, :])
                    nc.scalar.dma_start(hs[:, 2:4, :], h_src[:, 2:4, :])
                else:
                    nc.sync.dma_start(hs[:], h_src)
                if not widfirst and not v2:
                    wid_t = wpool.tile([128, KS], i32, tag="wid")
                    nc.sync.dma_start(wid_t[:], wid_d[i].rearrange("(p k) -> p k", k=KS))
                if not v2:
                    wid_f = wpool.tile([128, KS], f32, tag="widf")
                    nc.vector.tensor_copy(wid_f[:], wid_t[:])

                # ---- one-hot [s, t] ----
                oh = wpool.tile([128, KS, T], f32r, tag="oh")
                for k in range(KS):
                    wsrc = wid_f_all[:, i, k : k + 1] if v2 else wid_f[:, k : k + 1]
                    eng = nc.gpsimd if k < ohpool else nc.vector
                    eng.tensor_scalar(
                        oh[:, k, :], iota_f[:], wsrc, None, Alu.is_equal
                    )

                if v2 == 1 and v3:
                    # ---- single k-reduction; counts matmuls emitted after mm1
                    # so PE never stalls waiting on the DVE reduce ----
                    ohsum = wpool.tile([128, T], f32, tag="ohsum")
                    nc.gpsimd.tensor_tensor(
                        ohsum[:], oh[:, 0, :].bitcast(f32), oh[:, 1, :].bitcast(f32),
                        Alu.add,
                    )
                    nc.gpsimd.tensor_tensor(
                        ohsum[:], ohsum[:], oh[:, 2, :].bitcast(f32), Alu.add
                    )
                    nc.gpsimd.tensor_tensor(
                        ohsum[:], ohsum[:], oh[:, 3, :].bitcast(f32), Alu.add
                    )
                elif v2 == 1:
                    # ---- counts^T[t, 1] per c-chunk, straight to column form ----
                    counts_ps = ppool.tile([128, CT, 2], f32, tag="cnt")
                    if "cnt" in ablated:
                        nc.vector.memset(counts_ps[:], 2.0)
                    for c in range(CT if "cnt" not in ablated else 0):
                        for k in range(KS):
                            nc.tensor.matmul(
                                counts_ps[:, c, :],
                                oh[:, k, c * 128 : (c + 1) * 128],
                                ones2[:],
                                start=(k == 0),
                                stop=(k == KS - 1),
                            )
                if v2 == 1 and not v3:
                    # ---- scol[t] = (t < len) / max(counts, 1), per-partition ----
                    crec = wpool.tile([128, CT], f32, tag="crec")
                    nc.vector.tensor_scalar(
                        crec[:], counts_ps[:, :, 0], 1.0, None, Alu.max
                    )
                    nc.vector.reciprocal(crec[:], crec[:])
                    scol = wpool.tile([128, CT], f32, tag="scol")
                    nc.vector.scalar_tensor_tensor(
                        scol[:], iota_col[:], tl_bc[:, i : i + 1], crec[:],
                        Alu.is_lt, Alu.mult,
                    )
                else:
                    # ---- counts[1, t] = sum_s oh[s, t] ----
                    counts_ps = ppool.tile([1, T], f32, tag="cnt")
                    if "cnt" in ablated:
                        nc.vector.memset(counts_ps[:], 2.0)
                    for k in range(KS if "cnt" not in ablated else 0):
                        nc.tensor.matmul(
                            counts_ps[:],
                            ones_col[:],
                            oh[:, k, :],
                            start=(k == 0),
                            stop=(k == KS - 1),
                        )

                    # ---- scale[t] = (t < len) / max(counts, 1) ----
                    cmax = wpool.tile([1, T], f32, tag="cmax")
                    nc.vector.tensor_scalar(cmax[:], counts_ps[:], 1.0, None, Alu.max)
                    crec = wpool.tile([1, T], f32, tag="crec")
                    nc.vector.reciprocal(crec[:], cmax[:])
                    mask = wpool.tile([1, T], f32, tag="mask")
                    nc.vector.tensor_scalar(
                        mask[:], iota_f[0:1, :], tl_f[0:1, i : i + 1], None, Alu.is_lt
                    )
                    srow = wpool.tile([1, T], f32, tag="srow")
                    nc.vector.tensor_tensor(srow[:], crec[:], mask[:], Alu.mult)
                    # transpose scale_row -> per-partition scale columns [128, CT]
                    scol = wpool.tile([128, CT], f32, tag="scol")
                    scol_eng = nc.scalar if widfirst else nc.sync
                    for c in range(CT):
                        scol_eng.dma_start(
                            scol[:, c : c + 1],
                            srow[0:1, c * 128 : (c + 1) * 128],
                        )

                # ---- matmul 1: pooled_sums^T [h, t] ----
                if v3 == 2:
                    pt_ps = [
                        ppool.tile([128, 4 * T], f32, name="ptA", tag="ptA"),
                        ppool.tile([128, 2 * T], f32, name="ptB", tag="ptB"),
                    ]
                else:
                    pt_ps = [
                        ppool.tile([128, 2 * T], f32, name=f"pt{j}", tag=f"pt{j}")
                        for j in range(3)
                    ]
                for m in range(KH if "mm1" not in ablated else 0):
                    if v3 == 2:
                        dst = (
                            pt_ps[0][:, m * T : (m + 1) * T]
                            if m < 4
                            else pt_ps[1][:, (m - 4) * T : (m - 3) * T]
                        )
                    else:
                        dst = pt_ps[m // 2][:, (m % 2) * T : (m % 2 + 1) * T]
                    for k in range(KS):
                        nc.tensor.matmul(
                            dst,
                            hs[:, k, m * 128 : (m + 1) * 128],
                            oh[:, k, :],
                            start=(k == 0),
                            stop=(k == KS - 1),
                        )
                # ---- move to SBUF (plain copies, split ACT/DVE) ----
                pts = wpool.tile([128, KH, T], f32r, tag="pts")
                if "mm1" in ablated:
                    for j in range(3):
                        nc.vector.memset(pt_ps[j][:], 0.5)
                if v2 and v3 == 2:
                    # two merged copies: ACT takes the 4-chunk tile, DVE the rest
                    nc.scalar.copy(pts[:, 0:4, :], pt_ps[0][:, 0 : 4 * T])
                    nc.vector.tensor_copy(pts[:, 4:6, :], pt_ps[1][:, 0 : 2 * T])
                elif v2:
                    # merged pair copies [128, 2T]: ACT, DVE, ACT
                    for j in range(3):
                        src_ap = pt_ps[j][:, 0 : 2 * T]
                        dst_ap = pts[:, 2 * j : 2 * j + 2, :]
                        if j == 1:
                            nc.vector.tensor_copy(dst_ap, src_ap)
                        else:
                            nc.scalar.copy(dst_ap, src_ap)
                else:
                    for m in range(KH):
                        src_ap = pt_ps[m // 2][:, (m % 2) * T : (m % 2 + 1) * T]
                        if m % 2 == 0 and not ptsdve:
                            nc.scalar.copy(pts[:, m, :], src_ap)
                        else:
                            nc.vector.tensor_copy(pts[:, m, :], src_ap)

                # ---- matmul 2: out[t, e] = pooled @ W^T ----
                out_ps = [
                    ppool.tile([128, E], f32, name=f"o2{c}", tag=f"o2{c}", bufs=psum_bufs)
                    for c in range(CT)
                ]
                if "mm2" in ablated:
                    for c in range(CT):
                        nc.vector.memset(out_ps[c][:], 0.25)
                for c in range(CT if "mm2" not in ablated else 0):
                    for k in range(KH):
                        nc.tensor.matmul(
                            out_ps[c][:],
                            pts[:, k, c * 128 : (c + 1) * 128],
                            wt_t[:, k, :],
                            start=(k == 0),
                            stop=(k == KH - 1),
                        )
                if v2 == 1 and v3:
                    # counts emitted LAST on PE: the DVE k-reduce has had
                    # mm1+mm2 time to land, so PE never stalls on it
                    counts_ps = ppool.tile([128, CT, 2], f32, tag="cnt")
                    for c in range(CT):
                        nc.tensor.matmul(
                            counts_ps[:, c, :],
                            ohsum[:, c * 128 : (c + 1) * 128].bitcast(f32r),
                            ones2[:],
                            start=True,
                            stop=True,
                        )
                    crec = wpool.tile([128, CT], f32, tag="crec")
                    nc.vector.tensor_scalar(
                        crec[:], counts_ps[:, :, 0], 1.0, None, Alu.max
                    )
                    nc.vector.reciprocal(crec[:], crec[:])
                    scol = wpool.tile([128, CT], f32, tag="scol")
                    nc.vector.scalar_tensor_tensor(
                        scol[:], iota_col[:], tl_bc[:, i : i + 1], crec[:],
                        Alu.is_lt, Alu.mult,
                    )

                # ---- scale (per-partition), add bias, PSUM -> SBUF -> DRAM ----
                if pairld:
                    outs = outs2[:, i % 2]
                else:
                    outs = wpool.tile([128, CT, E], f32, tag="outs")
                if v2:
                    for c in range(CT):
                        nc.vector.scalar_tensor_tensor(
                            outs[:, c, :], out_ps[c][:], scol[:, c : c + 1], b_bc[:],
                            Alu.mult, Alu.add,
                        )
                else:
                    for c in range(CT):
                        nc.scalar.activation(
                            outs[:, c, :],
                            out_ps[c][:],
                            Act.Copy,
                            scale=scol[:, c : c + 1],
                        )
                        nc.vector.tensor_tensor(outs[:, c, :], outs[:, c, :], b_bc[:], Alu.add)
                if "st" in ablated:
                    pass
                elif pairld:
                    # one store per pair: 512 descriptors of 2KB, opposite
                    # ring from the pair's load
                    if i % 2 == 1:
                        steng = nc.scalar if (i // 2) % 2 == 0 else nc.sync
                        st_pat = "b (p c) e -> p b c e" if tperm else "b (c p) e -> p b c e"
                        steng.dma_start(
                            out_d[i - 1 : i + 1].rearrange(st_pat, c=CT),
                            outs2[:],
                        )
                elif outmerge:
                    eng = nc.scalar if i % 2 else nc.sync
                    eng.dma_start(
                        out_d[i].rearrange("(c p) e -> p c e", c=CT), outs[:, :, :]
                    )
                else:
                    for c in range(CT):
                        if dmav:
                            eng = nc.vector if c == 0 else nc.scalar
                        else:
                            eng = nc.scalar if (dma2 and c == 1) else nc.sync
                        eng.dma_start(out_d[i, c * 128 : (c + 1) * 128, :], outs[:, c, :])

    nc.compile()
    return nc


def _get_nc(reps: int = 1, **opts):
    key = f"nc{reps}|{sorted(opts.items())}"
    if key not in _cache:
        _cache[key] = _build(reps, **opts)
    return _cache[key]


def _in_maps(hidden_states, word_ids, token_lengths, W, b):
    wt = np.ascontiguousarray(W.T.astype(np.float32, copy=False))
    bias = np.ascontiguousarray(b.astype(np.float32, copy=False)).reshape(1, E)
    maps = []
    for c in range(NCORES):
        sl = slice(c * BL, (c + 1) * BL)
        maps.append(
            {
                "h": np.ascontiguousarray(hidden_states[sl]).astype(np.float32, copy=False),
                "wid": np.ascontiguousarray(word_ids[sl]).astype(np.int32, copy=False),
                "tl": np.ascontiguousarray(token_lengths[sl]).astype(np.int32, copy=False).reshape(1, BL),
                "wt": wt,
                "bias": bias,
                "ones": np.ones((1, 128), np.float32),
            }
        )
    return maps


def kernel(hidden_states, word_ids, token_lengths, W, b):
    from concourse import bass_utils

    nc = _get_nc()
    maps = _in_maps(hidden_states, word_ids, token_lengths, W, b)
    res = bass_utils.run_bass_kernel_spmd(nc, maps, core_ids=list(range(NCORES)))
    out = np.concatenate([res.results[c]["out"] for c in range(NCORES)], axis=0)
    return out

